# revision 1
# baseline (speedup 1.0000x reference)
"""Distributed multi-head attention kernel for 8 TRN2 NeuronCores.

Sharding: core c handles batch b = c//2 and head-group hg = c%2 (4 of 8
heads = 256 output columns).  Output slices are disjoint -> no collectives;
the host concatenates the 8 slices (bf16 device output, f32 host).

Device algorithm (per core), bf16 matmuls / f32 softmax; the steady state
is paced by the Scalar/ACT engine's exp stream (~1.15us per key chunk), so
everything else is scheduled to hide under it:
  - host permutes the key axis (unmasked keys first, ascending) and
    transposes inputs to [D, S]; only the first NU=ceil(max_unmasked/128)
    key chunks enter scores/exp/PV (sparse attention over v_mask)
  - DMA rules learned from traces: an engine queue's compute instructions
    inherit waits on DMAs issued from the same queue, so the scalar ring
    carries only first-pass feeds (landing before the first exp) and the
    sync ring the rest; side data + iota ride gpsimd-SWDGE (small only --
    a large SWDGE transfer starves the HWDGE rings of SDMA engines); vt
    loads go first on both rings so the V projections don't stagger
  - scores in S^T layout [k', q]; the two heads of a pair use PE row
    groups 0-63 / 64-127 so their score matmuls run concurrently; one
    [128, 1024] PSUM tile holds both heads' scores for a q-tile and a
    single ACT exp (per-partition key bias; scale=0.125) covers both
  - causal masking: block-level skips from a union-over-batches liveness
    structure (SPMD-identical graph); straddling blocks are trimmed to
    q >= qlo (union) in scores/exp/mask/PV, and the residual staircase
    masks are generated on device (gpsimd iota + vector is_ge against
    per-band thresholds) -- no mask bytes cross HBM
  - PV: O^T[65, q] accumulated in PSUM over key chunks; row 64 (ones
    column appended to VW) is the softmax denominator
  - the 8 (q-tile t, head-pair dc) passes are emitted t-major; each
    pass's K/Q/V projections are emitted as the previous pass begins
    (proj psum shares the score pool, so they must not interleave with
    the chunk stream); q-tile proj copies ride the scalar queue (idle
    exactly while the next scores are missing), k/v copies the vector
    queue; pass finalize is deferred to the next pass's third chunk so
    the PE never stalls on the psO->SBUF copy at pass ends
  - dead queries (all causally-allowed keys masked): host precomputes
    fvec = v_perm^T @ F; 16 tiny matmuls add the fix into output columns
    0..3, with dead-slot counts joined at finalize
  - finalize per pass: DVE-copy psO to SBUF (bf16), PE-transpose to
    [q, 65], scale by q_mask/rowsum, per-(t,dc) bf16 output DMA on sync
"""

import numpy as np
import ml_dtypes

BF = ml_dtypes.bfloat16
B, S, D = 4, 2048, 512
HG = 256          # output columns per core (4 heads x 64)
KS = 65           # head value width + ones column
NCH = 16          # total key chunks of 128
NEG = np.float32(-1e10)

_CACHE = {}


def _structure(v_mask):
    """Key permutations + block liveness (union over batches -> SPMD-safe)."""
    perms, n1s = [], []
    for b in range(B):
        unm = np.where(v_mask[b] == 1)[0]
        msk = np.where(v_mask[b] == 0)[0]
        perms.append(np.concatenate([unm, msk]))
        n1s.append(len(unm))
    NU = int(max(-(-n // 128) for n in n1s))
    live = set()
    band = set()
    qlo_raw = {}
    for b in range(B):
        unm = perms[b][:n1s[b]]
        for c in range(NU):
            seg = unm[128 * c:min(128 * (c + 1), n1s[b])]
            if len(seg) == 0:
                continue
            lo, hi = int(seg[0]), int(seg[-1])
            for t in range(4):
                if lo > 512 * t + 511:
                    continue
                live.add((c, t))
                ql = max(0, lo - 512 * t)
                qlo_raw[(c, t)] = min(qlo_raw.get((c, t), 512), ql)
                if hi > 512 * t:
                    band.add((c, t))
    live_lists = tuple(tuple(sorted(c for (c, tt) in live if tt == t))
                       for t in range(4))
    band_list = tuple(sorted(band))
    # queries below qlo see no key of the chunk (union over batches); the
    # first live chunk of each tile keeps full width (starts the psO group)
    qlo = {}
    for (c, t), v in qlo_raw.items():
        qlo[(c, t)] = 0 if c == live_lists[t][0] else (v // 8) * 8
    qlo_t = tuple(sorted(qlo.items()))
    return perms, n1s, NU, live_lists, band_list, qlo_t


def _build(NU, live_lists, band_list, qlo_t):
    import concourse.bass as bass  # noqa: F401
    from concourse import bacc
    import concourse.mybir as mybir
    from concourse.tile import TileContext

    F32 = mybir.dt.float32
    BF16 = mybir.dt.bfloat16
    I32 = mybir.dt.int32
    Exp = mybir.ActivationFunctionType.Exp
    nband = len(band_list)
    band_idx = {ct: i for i, ct in enumerate(band_list)}
    qlo = dict(qlo_t)
    kp_tiles = -(-NU * 128 // 512)  # s-tiles of K to project

    nc = bacc.Bacc()
    qT = nc.declare_dram_parameter("qT", [D, S], BF16, isOutput=False)
    kT = nc.declare_dram_parameter("kT", [D, S], BF16, isOutput=False)
    vT = nc.declare_dram_parameter("vT", [D, S], BF16, isOutput=False)
    wall = nc.declare_dram_parameter("wall", [D, 3 * HG], BF16, isOutput=False)
    vbias = nc.declare_dram_parameter("vbias", [128, NCH], F32, isOutput=False)
    qmask = nc.declare_dram_parameter("qmask", [128, NCH], F32, isOutput=False)
    bthr = nc.declare_dram_parameter("bthr", [128, nband], F32, isOutput=False)
    fvec = nc.declare_dram_parameter("fvec", [128, 16], BF16, isOutput=False)
    cnt = nc.declare_dram_parameter("cnt", [128, 4], F32, isOutput=False)
    ident = nc.declare_dram_parameter("ident", [128, 128], BF16, isOutput=False)
    out = nc.declare_dram_parameter("out", [S, HG], BF16, isOutput=True)

    with TileContext(nc) as tc:
        with tc.tile_pool(name="sb", bufs=1) as sb, \
             tc.tile_pool(name="ps", bufs=1, space="PSUM") as ps:

            def sbt(name, shape, dtype, bufs=1, tag=None):
                return sb.tile(shape, dtype, name=name, tag=tag or name, bufs=bufs)

            # input tiles first; loads stream in column halves on both HWDGE queues
            def decl_xT(pfx):
                return [sb.tile([128, S], BF16, name=f"{pfx}xT{Dc}",
                                tag=f"{pfx}xT{Dc}", bufs=1) for Dc in range(4)]

            vt = decl_xT("v")
            kt = decl_xT("k")
            qt = decl_xT("q")
            klim = NU * 128

            # CRITICAL DMA RULE: compute instructions on a queue wait for ALL
            # DMAs previously issued from that queue (coarse aggregated
            # semaphores).  The scalar queue therefore issues NO input DMAs
            # (it runs the exp stream); sync and gpsimd-SWDGE split the bulk.
            w_sb = {}
            wall_sb = []
            for Dc in range(4):
                tw = sbt(f"wall{Dc}", [128, 3 * HG], BF16)
                wall_sb.append(tw)
                for j, nm in enumerate(("q", "k", "v")):
                    w_sb[(nm, Dc)] = tw[:, HG * j:HG * (j + 1)]

            # gpsimd queue: side data + iota first (so iota's implicit wait
            # covers only the tiny transfers), then the bulk vt / qt tail
            bthr_sb = sbt("bthr_sb", [128, nband], F32)
            nc.gpsimd.dma_start(out=bthr_sb, in_=bthr[:])
            qmask_sb = sbt("qmask_sb", [128, NCH], F32)
            nc.gpsimd.dma_start(out=qmask_sb, in_=qmask[:])
            fvec_sb = sbt("fvec_sb", [128, 16], BF16)
            nc.gpsimd.dma_start(out=fvec_sb, in_=fvec[:])
            cnt_sb = sbt("cnt_sb", [128, 4], F32)
            nc.gpsimd.dma_start(out=cnt_sb, in_=cnt[:])
            ident_sb = sbt("ident_sb", [128, 128], BF16)
            nc.gpsimd.dma_start(out=ident_sb, in_=ident[:])
            iota_sb = sbt("iota_sb", [128, 512], I32)
            nc.gpsimd.iota(iota_sb, [[1, 512]], channel_multiplier=0)

            # first-pass feeds split across both HWDGE rings, consumption-
            # ordered (vt first so the V projections don't stagger); the
            # scalar ring's last input DMA lands before the first exp so the
            # exp stream never inherits a late aggregated DMA wait
            def ld(eng, tiles, c0, c1, dram, Dcs):
                for Dc in Dcs:
                    eng.dma_start(out=tiles[Dc][:, c0:c1],
                                  in_=dram[128 * Dc:128 * (Dc + 1), c0:c1])

            ld(nc.scalar, vt, 0, klim, vT, (0, 1))
            ld(nc.sync, vt, 0, klim, vT, (2, 3))
            for Dc in (0, 1):
                nc.scalar.dma_start(out=wall_sb[Dc],
                                    in_=wall[128 * Dc:128 * (Dc + 1), :])
            for Dc in (2, 3):
                nc.sync.dma_start(out=wall_sb[Dc],
                                  in_=wall[128 * Dc:128 * (Dc + 1), :])
            vbias_sb = sbt("vbias_sb", [128, NCH], F32)
            nc.sync.dma_start(out=vbias_sb, in_=vbias[:])
            ld(nc.scalar, kt, 0, klim, kT, (0, 1))
            ld(nc.sync, kt, 0, klim, kT, (2, 3))
            ld(nc.scalar, qt, 0, 512, qT, (0, 1))
            ld(nc.sync, qt, 0, 512, qT, (2, 3))
            ld(nc.sync, qt, 512, 1024, qT, (0, 1, 2, 3))
            ld(nc.sync, qt, 1024, 2048, qT, (0, 1, 2, 3))
            bmask_sb = sbt("bmask_sb", [128, nband * 512], BF16)
            bdone = set()

            def ensure_bmask(t):
                for i, (c, tt) in enumerate(band_list):
                    if tt == t and i not in bdone:
                        bdone.add(i)
                        nc.vector.tensor_scalar(
                            bmask_sb[:, 512 * i:512 * (i + 1)], iota_sb,
                            bthr_sb[:, i:i + 1], None, mybir.AluOpType.is_ge)

            qwT = [sbt(f"qwT{i}", [128, S], BF16) for i in range(2)]
            kwT = [sbt(f"kwT{i}", [128, S], BF16) for i in range(2)]
            vw = [sbt(f"vw{i}", [128, 4 * KS], BF16) for i in range(NU)]

            def vproj(st):
                p = ps.tile([128, HG], F32, name="pprj", tag="psS", bufs=2)
                for Dc in range(4):
                    nc.tensor.matmul(p, vt[Dc][:, 128 * st:128 * (st + 1)],
                                     w_sb[("v", Dc)], start=(Dc == 0), stop=(Dc == 3))
                t = vw[st]
                nc.vector.memset(
                    t.rearrange("p (h j) -> p h j", j=KS)[:, :, 64:65], 1.0)
                nc.vector.tensor_copy(
                    t.rearrange("p (h j) -> p h j", j=KS)[:, :, 0:64],
                    p.rearrange("p (h j) -> p h j", j=64))

            def proj_kq(dc, which, st2):
                xt, dst, wnm = ((kt, kwT, "k") if which == "k" else (qt, qwT, "q"))
                lim = klim if which == "k" else S
                w = min(512, lim - 512 * st2)
                p = ps.tile([128, 512], F32, name="pprj2", tag="psS", bufs=2)
                for Dc in range(4):
                    nc.tensor.matmul(
                        p[:, 0:w], w_sb[(wnm, Dc)][:, 128 * dc:128 * (dc + 1)],
                        xt[Dc][:, 512 * st2:512 * st2 + w],
                        start=(Dc == 0), stop=(Dc == 3))
                # only the q-tile copy gates the next pass's first scores:
                # it rides the scalar queue (idle exactly then); k copies
                # have chunks of slack and stay on the vector queue
                if which == "q":
                    nc.scalar.copy(dst[dc][:, 512 * st2:512 * st2 + w],
                                   p[:, 0:w])
                else:
                    nc.vector.tensor_copy(dst[dc][:, 512 * st2:512 * st2 + w],
                                          p[:, 0:w])

            # projection work for pass p+1 is spread through pass p's chunk
            # stream (one op per chunk) so the PE fills ACT-paced slack and
            # no projection burst starves the exp stream at pass boundaries;
            # pass (0,0)'s needs run upfront while the input lands
            vdone = [0]
            kdone = [0, 0]
            qdone = [0, 0]

            def proj_needs(t, dc):
                lst = []
                lc = live_lists[t]
                while vdone[0] < lc[-1] + 1:
                    st = vdone[0]
                    lst.append(lambda st=st: vproj(st))
                    vdone[0] += 1
                need_k = min(kp_tiles, -(-(128 * (lc[-1] + 1)) // 512))
                while kdone[dc] < need_k:
                    s = kdone[dc]
                    lst.append(lambda dc=dc, s=s: proj_kq(dc, "k", s))
                    kdone[dc] += 1
                while qdone[dc] < t + 1:
                    s = qdone[dc]
                    lst.append(lambda dc=dc, s=s: proj_kq(dc, "q", s))
                    qdone[dc] += 1
                return lst

            # ---- attention: q-tile passes, dc-interleaved, compacted keys ----
            # finalize of pass p is emitted after pass p+1's chunk stream so
            # the PE never stalls on the DVE psO->SBUF copy at pass ends
            ofin = sbt("ofin", [128, NCH * HG], BF16)

            def make_finalize(t, dc, psO):
                def fin():
                    h0, h1 = 2 * dc, 2 * dc + 1
                    for hh in (h0, h1):
                        ot = sb.tile([KS, 512], BF16, name="ot", tag="ot", bufs=3)
                        nc.vector.tensor_copy(ot, psO[hh])
                        tp = ps.tile([128, 4 * 66], BF16, name="tp", tag="psO",
                                     bufs=4)
                        for j in range(4):
                            nc.tensor.matmul(tp[:, 66 * j:66 * j + KS],
                                             ot[:, 128 * j:128 * (j + 1)],
                                             ident_sb[0:KS, 0:KS],
                                             is_transpose=True,
                                             start=(j == 0), stop=(j == 3),
                                             skip_group_check=True)
                        rs = sb.tile([128, 4], F32, name="rs", tag="rs", bufs=4)
                        if t == 0:
                            nc.vector.tensor_add(
                                rs.rearrange("p (j o) -> p j o", o=1),
                                tp.rearrange("p (j f) -> p j f", f=66)[:, :, 64:65],
                                cnt_sb.rearrange("p (j o) -> p j o", o=1))
                        else:
                            nc.vector.tensor_scalar_add(
                                rs.rearrange("p (j o) -> p j o", o=1),
                                tp.rearrange("p (j f) -> p j f", f=66)[:, :, 64:65],
                                1e-30)
                        rcp = sb.tile([128, 4], F32, name="rcp", tag="rcp", bufs=4)
                        nc.vector.reciprocal(rcp, rs)
                        scl = sb.tile([128, 4], F32, name="scl", tag="scl", bufs=4)
                        nc.vector.tensor_mul(scl, rcp, qmask_sb[:, 4 * t:4 * (t + 1)])
                        for j in range(4):
                            col = (4 * t + j) * HG + 64 * hh
                            nc.vector.tensor_scalar_mul(
                                ofin[:, col:col + 64], tp[:, 66 * j:66 * j + 64],
                                scl[:, j:j + 1])
                    nc.sync.dma_start(
                        out=out.rearrange("(j p) n -> p j n", p=128)
                        [:, 4 * t:4 * (t + 1), 128 * dc:128 * (dc + 1)],
                        in_=ofin.rearrange("p (j n) -> p j n", n=HG)
                        [:, 4 * t:4 * (t + 1), 128 * dc:128 * (dc + 1)])
                return fin

            passes = [(t, dc) for t in range(4) for dc in range(2)]
            for f in proj_needs(0, 0):
                f()
            ensure_bmask(0)

            pending = None
            prework = []
            for pi, (t, dc) in enumerate(passes):
                    h0, h1 = 2 * dc, 2 * dc + 1
                    kw_t, qw_t = kwT[dc], qwT[dc]
                    # prework = next pass's projections, spread one-per-chunk
                    # through this pass (chunk 0 is left to the deferred
                    # finalize; leftovers flush at pass end)
                    if pi + 1 < len(passes):
                        tn, dcn = passes[pi + 1]
                        prework = proj_needs(tn, dcn)
                        if dcn == 0:
                            prework.append(lambda tn=tn: ensure_bmask(tn))
                    else:
                        prework = []
                    lc = live_lists[t]
                    per = -(-len(prework) // max(1, len(lc) - 1))
                    fin_at = min(2, len(lc) - 1)
                    psO = {}
                    for hh in (h0, h1):
                        psO[hh] = ps.tile([KS, 512], F32, name=f"psO{hh}",
                                          tag="psO", bufs=4)
                    ci = 0
                    for c in range(lc[-1] + 1):
                        if c in lc:
                            o = qlo.get((c, t), 0)
                            psS = ps.tile([128, 1024], F32, name="psS",
                                          tag="psS", bufs=2)
                            for i, ho in enumerate((0, 64)):
                                nc.tensor.matmul(
                                    psS[:, 512 * i + o:512 * (i + 1)],
                                    kw_t[ho:ho + 64, 128 * c:128 * (c + 1)],
                                    qw_t[ho:ho + 64, 512 * t + o:512 * (t + 1)],
                                    start=True, stop=True)
                            U = sb.tile([128, 1024], BF16, name="U", tag="U",
                                        bufs=8)
                            nc.scalar.activation(
                                U.rearrange("p (i q) -> p i q", q=512)[:, :, o:],
                                psS.rearrange("p (i q) -> p i q", q=512)[:, :, o:],
                                Exp, bias=vbias_sb[:, c:c + 1], scale=0.125)
                            for i, hh in enumerate((h0, h1)):
                                Ui = U[:, 512 * i + o:512 * (i + 1)]
                                if (c, t) in band_idx:
                                    off = band_idx[(c, t)] * 512
                                    nc.vector.tensor_mul(
                                        Ui, Ui, bmask_sb[:, off + o:off + 512])
                                stop = (c == lc[-1]) if t > 0 else False
                                nc.tensor.matmul(psO[hh][:, o:],
                                                 vw[c][:, KS * hh:KS * (hh + 1)],
                                                 Ui,
                                                 start=(c == lc[0]), stop=stop,
                                                 skip_group_check=True)
                            if ci == fin_at and pending is not None:
                                pending()
                                pending = None
                            else:
                                for _ in range(per):
                                    if prework:
                                        prework.pop(0)()
                            ci += 1
                    while prework:
                        prework.pop(0)()
                    if t == 0:
                        # dead-query fix: psO[:, 0:4] += Wv_hh^T @ fvec
                        for hh in (h0, h1):
                            for Dc in range(4):
                                nc.tensor.matmul(
                                    psO[hh][0:64, 0:4],
                                    w_sb[("v", Dc)][:, 64 * hh:64 * (hh + 1)],
                                    fvec_sb[:, 4 * Dc:4 * (Dc + 1)],
                                    start=False, stop=(Dc == 3),
                                    skip_group_check=True)
                    pending = make_finalize(t, dc, psO)
            pending()

    nc.compile()
    return nc


def _prep_inputs(q, k, v, v_mask, q_mask, Wq, Wk, Wv, perms, n1s, band_list):
    q = np.asarray(q, np.float32)
    k = np.asarray(k, np.float32)
    v = np.asarray(v, np.float32)
    v_mask = np.asarray(v_mask, np.float32)
    q_mask = np.asarray(q_mask, np.float32)
    Wq = np.asarray(Wq, np.float32)
    Wk = np.asarray(Wk, np.float32)
    Wv = np.asarray(Wv, np.float32)
    ident = np.eye(128, dtype=np.float32)
    nband = len(band_list)

    in_maps = []
    for core in range(8):
        b, hg = core // 2, core % 2
        cs = slice(hg * HG, (hg + 1) * HG)
        perm, n1 = perms[b], n1s[b]
        vb = np.where(np.arange(S) < n1, np.float32(0), NEG).astype(np.float32)
        fix = np.zeros((S, 4), np.float32)
        if v_mask[b, 0] == 0:
            first_one = int(np.argmax(v_mask[b] > 0))
            ks_ = np.arange(S)
            for dj in range(min(first_one, 4)):
                sel = ((ks_ <= dj) & (v_mask[b] == 0)) | \
                      ((ks_ > dj) & (v_mask[b] == 1))
                fix[:, dj] = sel[perm].astype(np.float32)
        fvec = (v[b][perm].T @ fix).astype(np.float32)
        cnt = np.full((128, 4), np.float32(1e-30))
        cnt[0:4, 0] += fix.sum(axis=0)
        # per-band threshold: mask[k, q] = (q >= pos_k - 512 t)
        bthr = np.zeros((128, nband), np.float32)
        for i, (c, t) in enumerate(band_list):
            kpos = perm[128 * c:128 * (c + 1)].astype(np.float32)
            bthr[:, i] = kpos - 512.0 * t
        in_maps.append({
            "qT": np.ascontiguousarray(q[b].T).astype(BF),
            "kT": np.ascontiguousarray(k[b][perm].T).astype(BF),
            "vT": np.ascontiguousarray(v[b][perm].T).astype(BF),
            "wall": np.ascontiguousarray(
                np.concatenate([Wq[:, cs], Wk[:, cs], Wv[:, cs]],
                               axis=1)).astype(BF),
            "vbias": np.ascontiguousarray(vb.reshape(NCH, 128).T),
            "qmask": np.ascontiguousarray(q_mask[b].reshape(NCH, 128).T),
            "bthr": bthr,
            "fvec": np.ascontiguousarray(
                fvec.reshape(4, 128, 4).transpose(1, 0, 2)
                .reshape(128, 16)).astype(BF),
            "cnt": cnt,
            "ident": ident.astype(BF),
        })
    return in_maps


def kernel(q, k, v, v_mask, q_mask, Wq, Wk, Wv, _trace=False):
    from concourse.bass_utils import run_bass_kernel_spmd

    v_mask_f = np.asarray(v_mask, np.float32)
    perms, n1s, NU, live_lists, band_list, qlo_t = _structure(v_mask_f)
    key = (NU, live_lists, band_list, qlo_t)
    if _CACHE.get("key") != key:
        _CACHE["nc"] = _build(NU, live_lists, band_list, qlo_t)
        _CACHE["key"] = key
    nc = _CACHE["nc"]
    in_maps = _prep_inputs(q, k, v, v_mask, q_mask, Wq, Wk, Wv,
                           perms, n1s, band_list)
    res = run_bass_kernel_spmd(nc, in_maps, core_ids=list(range(8)), trace=_trace)
    _CACHE["last_result"] = res
    full = np.zeros((B, S, 2 * HG), np.float32)
    for core in range(8):
        b, hg = core // 2, core % 2
        full[b, :, hg * HG:(hg + 1) * HG] = np.asarray(
            res.results[core]["out"], np.float32)
    return full



# revision 3
# speedup vs baseline: 1.4654x; 1.4654x over previous
"""Distributed multi-head attention kernel for 8 TRN2 NeuronCores.

Sharding: core c handles batch b = c//2 and head-group hg = c%2 (4 of 8
heads = 256 output columns).  Output slices are disjoint -> no collectives;
the host concatenates the 8 slices (bf16 device output, f32 host).

Device algorithm (per core), bf16 matmuls / f32 softmax:
  - host compacts BOTH axes: keys permuted unmasked-first (sparse
    attention over v_mask) and queries compacted to q_mask==1 only
    (dead queries are exactly zero in the reference); only NU=
    ceil(max_unmasked/128) key chunks and NQ=ceil(max_live_q/512)
    query tiles enter the pipeline.  Causal-mask thresholds move to
    compacted index space via host-side searchsorted, so the device
    masking (iota + per-partition is_ge) is unchanged.
  - DMA rules learned from traces: an engine queue's compute
    instructions inherit waits on DMAs issued from the same queue, so
    the scalar ring carries only the first-pass feeds (landing before
    the first exp); the sync ring carries the rest, consumption-
    ordered; side data + iota ride gpsimd-SWDGE (small only)
  - scores in S^T layout [k', q]; the two heads of a pair use PE row
    groups 0-63 / 64-127 so their score matmuls run concurrently; one
    [128, 1024] PSUM tile holds both heads' scores for a q-tile and a
    single ACT exp (per-partition key bias; scale=0.125) covers both
  - causal masking: block-level skips from a union-over-batches
    liveness structure (SPMD-identical graph); straddling blocks are
    trimmed to q >= qlo (union) in scores/exp/mask/PV, and the
    residual staircase masks are generated on device (gpsimd iota +
    vector is_ge against per-band thresholds)
  - PV: O^T[65, q] accumulated in PSUM over key chunks; row 64 (ones
    column appended to VW) is the softmax denominator
  - the (q-tile t, head-pair dc) passes are emitted t-major; each
    pass's K/Q/V projections are spread through the previous pass's
    chunk stream (proj psum shares the score pool); q-tile proj copies
    ride the scalar queue, k/v copies the vector queue; pass finalize
    is deferred to the next pass's third chunk so the PE never stalls
    on the psO->SBUF copy at pass ends
  - dead queries (all causally-allowed keys masked but q_mask==1):
    host precomputes fvec = v_perm^T @ F at their compacted indices
    (they compact to the first columns); 16 tiny matmuls add the fix
    into output columns 0..3, with dead-slot counts joined at finalize
  - finalize per pass: DVE-copy psO to SBUF (bf16), PE-transpose to
    [q, 65], scale by q_mask/rowsum, per-(t,dc) bf16 output DMA on
    sync; host scatters compacted rows back to full [S, 512]
"""

import numpy as np
import ml_dtypes

BF = ml_dtypes.bfloat16
B, S, D = 4, 2048, 512
HG = 256          # output columns per core (4 heads x 64)
KS = 65           # head value width + ones column
NCH = 16          # total key chunks of 128
NEG = np.float32(-1e10)

_CACHE = {}


def _structure(v_mask, q_mask):
    """Key/query compaction + block liveness (union over batches)."""
    perms, n1s, qposs = [], [], []
    for b in range(B):
        unm = np.where(v_mask[b] == 1)[0]
        msk = np.where(v_mask[b] == 0)[0]
        perms.append(np.concatenate([unm, msk]))
        n1s.append(len(unm))
        qposs.append(np.where(q_mask[b] == 1)[0])
    NU = int(max(-(-n // 128) for n in n1s))
    NQ = int(max(-(-len(qp) // 512) for qp in qposs))
    live = set()
    band = set()
    qlo_raw = {}
    for b in range(B):
        unm = perms[b][:n1s[b]]
        qp = qposs[b]
        nq = len(qp)
        for c in range(NU):
            seg = unm[128 * c:min(128 * (c + 1), n1s[b])]
            if len(seg) == 0:
                continue
            lo, hi = int(seg[0]), int(seg[-1])
            # compacted index of the first query that sees lo / all of hi
            qlo_c = int(np.searchsorted(qp, lo))
            qhi_c = int(np.searchsorted(qp, hi))
            for t in range(NQ):
                tile_last = min(512 * (t + 1), nq) - 1
                if tile_last < 512 * t or qlo_c > tile_last:
                    continue
                live.add((c, t))
                ql = max(0, qlo_c - 512 * t)
                qlo_raw[(c, t)] = min(qlo_raw.get((c, t), 512), ql)
                if qhi_c > 512 * t:
                    band.add((c, t))
    live_lists = tuple(tuple(sorted(c for (c, tt) in live if tt == t))
                       for t in range(NQ))
    band_list = tuple(sorted(band))
    # queries below qlo see no key of the chunk (union over batches); the
    # first live chunk of each tile keeps full width (starts the psO group)
    qlo = {}
    for (c, t), v in qlo_raw.items():
        qlo[(c, t)] = 0 if c == live_lists[t][0] else (v // 8) * 8
    qlo_t = tuple(sorted(qlo.items()))
    return perms, n1s, qposs, NU, NQ, live_lists, band_list, qlo_t


def _build(NU, NQ, live_lists, band_list, qlo_t):
    import concourse.bass as bass  # noqa: F401
    from concourse import bacc
    import concourse.mybir as mybir
    from concourse.tile import TileContext

    F32 = mybir.dt.float32
    BF16 = mybir.dt.bfloat16
    I32 = mybir.dt.int32
    Exp = mybir.ActivationFunctionType.Exp
    nband = len(band_list)
    band_idx = {ct: i for i, ct in enumerate(band_list)}
    qlo = dict(qlo_t)
    klim = NU * 128
    NQT = NQ * 512
    kp_tiles = -(-klim // 512)  # s-tiles of K to project

    nc = bacc.Bacc()
    qT = nc.declare_dram_parameter("qT", [D, NQT], BF16, isOutput=False)
    kT = nc.declare_dram_parameter("kT", [D, S], BF16, isOutput=False)
    vT = nc.declare_dram_parameter("vT", [D, S], BF16, isOutput=False)
    wall = nc.declare_dram_parameter("wall", [D, 3 * HG], BF16, isOutput=False)
    vbias = nc.declare_dram_parameter("vbias", [128, NCH], F32, isOutput=False)
    qmask = nc.declare_dram_parameter("qmask", [128, 4 * NQ], F32, isOutput=False)
    bthr = nc.declare_dram_parameter("bthr", [128, nband], F32, isOutput=False)
    fvec = nc.declare_dram_parameter("fvec", [128, 16], BF16, isOutput=False)
    cnt = nc.declare_dram_parameter("cnt", [128, 4], F32, isOutput=False)
    ident = nc.declare_dram_parameter("ident", [128, 128], BF16, isOutput=False)
    out = nc.declare_dram_parameter("out", [NQT, HG], BF16, isOutput=True)

    with TileContext(nc) as tc:
        with tc.tile_pool(name="sb", bufs=1) as sb, \
             tc.tile_pool(name="ps", bufs=1, space="PSUM") as ps:

            def sbt(name, shape, dtype, bufs=1, tag=None):
                return sb.tile(shape, dtype, name=name, tag=tag or name, bufs=bufs)

            def decl_xT(pfx, w):
                return [sb.tile([128, w], BF16, name=f"{pfx}xT{Dc}",
                                tag=f"{pfx}xT{Dc}", bufs=1) for Dc in range(4)]

            vt = decl_xT("v", S)
            kt = decl_xT("k", S)
            qt = decl_xT("q", NQT)
            # v chunks needed by the first pass (tile 0's live chunks)
            v_first = min(klim, 128 * (live_lists[0][-1] + 1))

            w_sb = {}
            wall_sb = []
            for Dc in range(4):
                tw = sbt(f"wall{Dc}", [128, 3 * HG], BF16)
                wall_sb.append(tw)
                for j, nm in enumerate(("q", "k", "v")):
                    w_sb[(nm, Dc)] = tw[:, HG * j:HG * (j + 1)]

            # gpsimd queue: side data + iota (small transfers only)
            bthr_sb = sbt("bthr_sb", [128, nband], F32)
            nc.gpsimd.dma_start(out=bthr_sb, in_=bthr[:])
            qmask_sb = sbt("qmask_sb", [128, 4 * NQ], F32)
            nc.gpsimd.dma_start(out=qmask_sb, in_=qmask[:])
            fvec_sb = sbt("fvec_sb", [128, 16], BF16)
            nc.gpsimd.dma_start(out=fvec_sb, in_=fvec[:])
            cnt_sb = sbt("cnt_sb", [128, 4], F32)
            nc.gpsimd.dma_start(out=cnt_sb, in_=cnt[:])
            ident_sb = sbt("ident_sb", [128, 128], BF16)
            nc.gpsimd.dma_start(out=ident_sb, in_=ident[:])
            iota_sb = sbt("iota_sb", [128, 512], I32)
            nc.gpsimd.iota(iota_sb, [[1, 512]], channel_multiplier=0)

            # CRITICAL DMA RULE: compute instructions on a queue wait for ALL
            # DMAs previously issued from that queue (coarse aggregated
            # semaphores).  The scalar ring carries ONLY the first-pass feeds
            # (they land before the first exp); everything else rides sync.
            def ld(eng, tiles, c0, c1, dram, Dcs):
                for Dc in Dcs:
                    eng.dma_start(out=tiles[Dc][:, c0:c1],
                                  in_=dram[128 * Dc:128 * (Dc + 1), c0:c1])

            for Dc in (0, 1):
                nc.scalar.dma_start(out=wall_sb[Dc],
                                    in_=wall[128 * Dc:128 * (Dc + 1), :])
            for Dc in (2, 3):
                nc.sync.dma_start(out=wall_sb[Dc],
                                  in_=wall[128 * Dc:128 * (Dc + 1), :])
            vbias_sb = sbt("vbias_sb", [128, NCH], F32)
            nc.sync.dma_start(out=vbias_sb, in_=vbias[:])
            ld(nc.scalar, vt, 0, 128, vT, (0, 1))
            ld(nc.sync, vt, 0, 128, vT, (2, 3))
            ld(nc.scalar, kt, 0, 512, kT, (0, 1))
            ld(nc.sync, kt, 0, 512, kT, (2, 3))
            ld(nc.scalar, qt, 0, 512, qT, (0, 1))
            ld(nc.sync, qt, 0, 512, qT, (2, 3))
            # remaining bulk: sync only, consumption-ordered
            ld(nc.sync, vt, 128, v_first, vT, (0, 1, 2, 3))
            if v_first < klim:
                ld(nc.sync, vt, v_first, klim, vT, (0, 1, 2, 3))
            if klim > 512:
                ld(nc.sync, kt, 512, klim, kT, (0, 1, 2, 3))
            if NQT > 512:
                ld(nc.sync, qt, 512, NQT, qT, (0, 1, 2, 3))

            bmask_sb = sbt("bmask_sb", [128, nband * 512], BF16)
            bdone = set()

            def ensure_bmask(t):
                for i, (c, tt) in enumerate(band_list):
                    if tt == t and i not in bdone:
                        bdone.add(i)
                        nc.vector.tensor_scalar(
                            bmask_sb[:, 512 * i:512 * (i + 1)], iota_sb,
                            bthr_sb[:, i:i + 1], None, mybir.AluOpType.is_ge)

            qwT = [sbt(f"qwT{i}", [128, NQT], BF16) for i in range(2)]
            kwT = [sbt(f"kwT{i}", [128, S], BF16) for i in range(2)]
            vw = [sbt(f"vw{i}", [128, 4 * KS], BF16) for i in range(NU)]

            def vproj(st):
                p = ps.tile([128, HG], F32, name="pprj", tag="psS", bufs=2)
                for Dc in range(4):
                    nc.tensor.matmul(p, vt[Dc][:, 128 * st:128 * (st + 1)],
                                     w_sb[("v", Dc)], start=(Dc == 0), stop=(Dc == 3))
                t = vw[st]
                nc.vector.memset(
                    t.rearrange("p (h j) -> p h j", j=KS)[:, :, 64:65], 1.0)
                nc.vector.tensor_copy(
                    t.rearrange("p (h j) -> p h j", j=KS)[:, :, 0:64],
                    p.rearrange("p (h j) -> p h j", j=64))

            def proj_kq(dc, which, st2):
                xt, dst, wnm = ((kt, kwT, "k") if which == "k" else (qt, qwT, "q"))
                lim = klim if which == "k" else NQT
                w = min(512, lim - 512 * st2)
                p = ps.tile([128, 512], F32, name="pprj2", tag="psS", bufs=2)
                for Dc in range(4):
                    nc.tensor.matmul(
                        p[:, 0:w], w_sb[(wnm, Dc)][:, 128 * dc:128 * (dc + 1)],
                        xt[Dc][:, 512 * st2:512 * st2 + w],
                        start=(Dc == 0), stop=(Dc == 3))
                # only the q-tile copy gates the next pass's first scores:
                # it rides the scalar queue (idle exactly then); k copies
                # have chunks of slack and stay on the vector queue
                if which == "q":
                    nc.scalar.copy(dst[dc][:, 512 * st2:512 * st2 + w],
                                   p[:, 0:w])
                else:
                    nc.vector.tensor_copy(dst[dc][:, 512 * st2:512 * st2 + w],
                                          p[:, 0:w])

            # projection work for pass p+1 is spread through pass p's chunk
            # stream (one op per chunk) so the PE fills ACT-paced slack and
            # no projection burst starves the exp stream at pass boundaries;
            # only the minimal (k0, q0, v0) feed for pass (0,0) runs upfront
            vdone = [0]
            kdone = [0, 0]
            qdone = [0, 0]

            def proj_needs(t, dc):
                lst = []
                lc = live_lists[t]
                while vdone[0] < lc[-1] + 1:
                    st = vdone[0]
                    lst.append(lambda st=st: vproj(st))
                    vdone[0] += 1
                need_k = min(kp_tiles, -(-(128 * (lc[-1] + 1)) // 512))
                while kdone[dc] < need_k:
                    s = kdone[dc]
                    lst.append(lambda dc=dc, s=s: proj_kq(dc, "k", s))
                    kdone[dc] += 1
                while qdone[dc] < t + 1:
                    s = qdone[dc]
                    lst.append(lambda dc=dc, s=s: proj_kq(dc, "q", s))
                    qdone[dc] += 1
                return lst

            # ---- attention: q-tile passes, dc-interleaved, compacted keys ----
            # finalize of pass p is emitted after pass p+1's chunk stream so
            # the PE never stalls on the DVE psO->SBUF copy at pass ends
            ofin = sbt("ofin", [128, 4 * NQ * HG], BF16)

            def make_finalize(t, dc, psO):
                def fin():
                    h0, h1 = 2 * dc, 2 * dc + 1
                    for hh in (h0, h1):
                        ot = sb.tile([KS, 512], BF16, name="ot", tag="ot", bufs=3)
                        nc.vector.tensor_copy(ot, psO[hh])
                        tp = ps.tile([128, 4 * 66], BF16, name="tp", tag="psO",
                                     bufs=4)
                        for j in range(4):
                            nc.tensor.matmul(tp[:, 66 * j:66 * j + KS],
                                             ot[:, 128 * j:128 * (j + 1)],
                                             ident_sb[0:KS, 0:KS],
                                             is_transpose=True,
                                             start=(j == 0), stop=(j == 3),
                                             skip_group_check=True)
                        rs = sb.tile([128, 4], F32, name="rs", tag="rs", bufs=4)
                        if t == 0:
                            nc.vector.tensor_add(
                                rs.rearrange("p (j o) -> p j o", o=1),
                                tp.rearrange("p (j f) -> p j f", f=66)[:, :, 64:65],
                                cnt_sb.rearrange("p (j o) -> p j o", o=1))
                        else:
                            nc.vector.tensor_scalar_add(
                                rs.rearrange("p (j o) -> p j o", o=1),
                                tp.rearrange("p (j f) -> p j f", f=66)[:, :, 64:65],
                                1e-30)
                        rcp = sb.tile([128, 4], F32, name="rcp", tag="rcp", bufs=4)
                        nc.vector.reciprocal(rcp, rs)
                        scl = sb.tile([128, 4], F32, name="scl", tag="scl", bufs=4)
                        nc.vector.tensor_mul(scl, rcp, qmask_sb[:, 4 * t:4 * (t + 1)])
                        for j in range(4):
                            col = (4 * t + j) * HG + 64 * hh
                            nc.vector.tensor_scalar_mul(
                                ofin[:, col:col + 64], tp[:, 66 * j:66 * j + 64],
                                scl[:, j:j + 1])
                    nc.sync.dma_start(
                        out=out.rearrange("(j p) n -> p j n", p=128)
                        [:, 4 * t:4 * (t + 1), 128 * dc:128 * (dc + 1)],
                        in_=ofin.rearrange("p (j n) -> p j n", n=HG)
                        [:, 4 * t:4 * (t + 1), 128 * dc:128 * (dc + 1)])
                return fin

            passes = [(t, dc) for t in range(NQ) for dc in range(2)]
            # minimal upfront feed for pass (0,0): k tile 0, q tile 0, v chunk 0
            proj_kq(0, "k", 0)
            kdone[0] = 1
            proj_kq(0, "q", 0)
            qdone[0] = 1
            vproj(0)
            vdone[0] = 1
            leftovers = proj_needs(0, 0)
            ensure_bmask(0)

            pending = None
            prework = []
            for pi, (t, dc) in enumerate(passes):
                    h0, h1 = 2 * dc, 2 * dc + 1
                    kw_t, qw_t = kwT[dc], qwT[dc]
                    # prework = rest of this pass's feed (pass 0 only) + next
                    # pass's projections, spread one-per-chunk through this
                    # pass (chunk 2 is left to the deferred finalize;
                    # leftovers flush at pass end)
                    prework = leftovers
                    leftovers = []
                    if pi + 1 < len(passes):
                        tn, dcn = passes[pi + 1]
                        prework += proj_needs(tn, dcn)
                        if dcn == 0:
                            prework.append(lambda tn=tn: ensure_bmask(tn))
                    lc = live_lists[t]
                    per = -(-len(prework) // max(1, len(lc) - 1))
                    fin_at = min(2, len(lc) - 1)
                    psO = {}
                    for hh in (h0, h1):
                        psO[hh] = ps.tile([KS, 512], F32, name=f"psO{hh}",
                                          tag="psO", bufs=4)
                    ci = 0
                    for c in range(lc[-1] + 1):
                        if c in lc:
                            o = qlo.get((c, t), 0)
                            psS = ps.tile([128, 1024], F32, name="psS",
                                          tag="psS", bufs=2)
                            for i, ho in enumerate((0, 64)):
                                nc.tensor.matmul(
                                    psS[:, 512 * i + o:512 * (i + 1)],
                                    kw_t[ho:ho + 64, 128 * c:128 * (c + 1)],
                                    qw_t[ho:ho + 64, 512 * t + o:512 * (t + 1)],
                                    start=True, stop=True)
                            U = sb.tile([128, 1024], BF16, name="U", tag="U",
                                        bufs=8)
                            nc.scalar.activation(
                                U.rearrange("p (i q) -> p i q", q=512)[:, :, o:],
                                psS.rearrange("p (i q) -> p i q", q=512)[:, :, o:],
                                Exp, bias=vbias_sb[:, c:c + 1], scale=0.125)
                            for i, hh in enumerate((h0, h1)):
                                Ui = U[:, 512 * i + o:512 * (i + 1)]
                                if (c, t) in band_idx:
                                    off = band_idx[(c, t)] * 512
                                    nc.vector.tensor_mul(
                                        Ui, Ui, bmask_sb[:, off + o:off + 512])
                                stop = (c == lc[-1]) if t > 0 else False
                                nc.tensor.matmul(psO[hh][:, o:],
                                                 vw[c][:, KS * hh:KS * (hh + 1)],
                                                 Ui,
                                                 start=(c == lc[0]), stop=stop,
                                                 skip_group_check=True)
                            if ci == fin_at and pending is not None:
                                pending()
                                pending = None
                            else:
                                for _ in range(per):
                                    if prework:
                                        prework.pop(0)()
                            ci += 1
                    while prework:
                        prework.pop(0)()
                    if t == 0:
                        # dead-query fix: psO[:, 0:4] += Wv_hh^T @ fvec
                        for hh in (h0, h1):
                            for Dc in range(4):
                                nc.tensor.matmul(
                                    psO[hh][0:64, 0:4],
                                    w_sb[("v", Dc)][:, 64 * hh:64 * (hh + 1)],
                                    fvec_sb[:, 4 * Dc:4 * (Dc + 1)],
                                    start=False, stop=(Dc == 3),
                                    skip_group_check=True)
                    pending = make_finalize(t, dc, psO)
            pending()

    nc.compile()
    return nc


def _prep_inputs(q, k, v, v_mask, q_mask, Wq, Wk, Wv,
                 perms, n1s, qposs, NQ, band_list):
    q = np.asarray(q, np.float32)
    k = np.asarray(k, np.float32)
    v = np.asarray(v, np.float32)
    v_mask = np.asarray(v_mask, np.float32)
    Wq = np.asarray(Wq, np.float32)
    Wk = np.asarray(Wk, np.float32)
    Wv = np.asarray(Wv, np.float32)
    ident = np.eye(128, dtype=np.float32)
    nband = len(band_list)
    NQT = NQ * 512

    in_maps = []
    for core in range(8):
        b, hg = core // 2, core % 2
        cs = slice(hg * HG, (hg + 1) * HG)
        perm, n1, qp = perms[b], n1s[b], qposs[b]
        nq = len(qp)
        # compacted query order, padded with row 0 (masked off via qmask)
        qperm = np.concatenate([qp, np.zeros(NQT - nq, np.int64)])
        vb = np.where(np.arange(S) < n1, np.float32(0), NEG).astype(np.float32)
        qm_c = np.zeros(NQT, np.float32)
        qm_c[:nq] = 1.0
        # dead queries: pos < first unmasked key, q_mask==1; they compact
        # to the first columns of tile 0
        fix = np.zeros((S, 4), np.float32)
        cnt = np.full((128, 4), np.float32(1e-30))
        if v_mask[b, 0] == 0:
            first_one = int(np.argmax(v_mask[b] > 0))
            ks_ = np.arange(S)
            nd = int((qp < first_one).sum())
            for dj in range(min(nd, 4)):
                pj = int(qp[dj])
                sel = ((ks_ <= pj) & (v_mask[b] == 0)) | \
                      ((ks_ > pj) & (v_mask[b] == 1))
                fix[:, dj] = sel[perm].astype(np.float32)
        fvec = (v[b][perm].T @ fix).astype(np.float32)
        cnt[0:4, 0] += fix.sum(axis=0)
        # per-band threshold in compacted index space:
        # mask[k, qc] = (qc >= searchsorted(qp, pos_k) - 512 t)
        bthr = np.zeros((128, nband), np.float32)
        for i, (c, t) in enumerate(band_list):
            kpos = perm[128 * c:128 * (c + 1)]
            bthr[:, i] = np.searchsorted(qp, kpos).astype(np.float32) - 512.0 * t
        in_maps.append({
            "qT": np.ascontiguousarray(q[b][qperm].T).astype(BF),
            "kT": np.ascontiguousarray(k[b][perm].T).astype(BF),
            "vT": np.ascontiguousarray(v[b][perm].T).astype(BF),
            "wall": np.ascontiguousarray(
                np.concatenate([Wq[:, cs], Wk[:, cs], Wv[:, cs]],
                               axis=1)).astype(BF),
            "vbias": np.ascontiguousarray(vb.reshape(NCH, 128).T),
            "qmask": np.ascontiguousarray(qm_c.reshape(4 * NQ, 128).T),
            "bthr": bthr,
            "fvec": np.ascontiguousarray(
                fvec.reshape(4, 128, 4).transpose(1, 0, 2)
                .reshape(128, 16)).astype(BF),
            "cnt": cnt,
            "ident": ident.astype(BF),
        })
    return in_maps


def kernel(q, k, v, v_mask, q_mask, Wq, Wk, Wv, _trace=False):
    from concourse.bass_utils import run_bass_kernel_spmd

    v_mask_f = np.asarray(v_mask, np.float32)
    q_mask_f = np.asarray(q_mask, np.float32)
    perms, n1s, qposs, NU, NQ, live_lists, band_list, qlo_t = \
        _structure(v_mask_f, q_mask_f)
    key = (NU, NQ, live_lists, band_list, qlo_t)
    if _CACHE.get("key") != key:
        _CACHE["nc"] = _build(NU, NQ, live_lists, band_list, qlo_t)
        _CACHE["key"] = key
    nc = _CACHE["nc"]
    in_maps = _prep_inputs(q, k, v, v_mask, q_mask, Wq, Wk, Wv,
                           perms, n1s, qposs, NQ, band_list)
    res = run_bass_kernel_spmd(nc, in_maps, core_ids=list(range(8)), trace=_trace)
    _CACHE["last_result"] = res
    full = np.zeros((B, S, 2 * HG), np.float32)
    for core in range(8):
        b, hg = core // 2, core % 2
        o = np.asarray(res.results[core]["out"], np.float32)
        full[b, qposs[b], hg * HG:(hg + 1) * HG] = o[:len(qposs[b])]
    return full
